# revision 18
# baseline (speedup 1.0000x reference)
"""Chamfer distance 2D loss — Trainium2 Bass/Tile kernel.

Problem: pred/target [32, 2048, 2] f32. Per batch: 2048x2048 pairwise
squared-distance matrix, bidirectional min + mean, sqrt(sq+eps), mean over
batch -> scalar.

Strategy (per core, 4 of 32 batches, data parallel over 8 cores):
  - sq[i,j] = |p_i|^2 + |t_j|^2 - 2 p.t computed ENTIRELY on the TensorEngine
    as a K=10 fp16 matmul. fp16 hi/lo splitting (with power-of-2 row scaling
    to dodge fp16 denormal flush) keeps ~2^-24 effective input precision, so
    the catastrophic cancellation in the norm expansion is harmless while the
    PE streams at the full 1 col/cycle fp16 rate (fp32 would be 4x slower).
  - sqrt is monotonic => min-reduce the SQUARED distances; sqrt only on the
    reduced [128, 128] tile at the end.
  - Per (batch, pred-chunk) tile [128 preds x 2048 targets] in PSUM:
      * forward rowmin via DVE tensor_tensor_reduce (fold halves + min-accum)
      * PSUM->SBUF fp16 eviction on the Scalar engine (or fused into the
        rowmin via ttr straight from PSUM for a few tiles, to balance load)
      * backward direction: elementwise-min tree over the 16 evicted tiles
        (split between GPSIMD and DVE), then DMA-transpose + free-dim min.
  - Final: sqrt(min+eps) on ACT, row sums on DVE, partition sum via a
    ones-matmul -> [8,1] per-core output (4 fwd sums, 4 bwd sums).
Host sums the 8 cores' partials exactly as the reference mean does.
"""

import os
import sys
from contextlib import ExitStack

import numpy as np

for _p in ("/opt/trn_rl_repo", "/root/.axon_site/_ro/trn_rl_repo"):
    if os.path.isdir(_p) and _p not in sys.path:
        sys.path.insert(0, _p)

import concourse.bass as bass
import concourse.tile as tile
from concourse import bacc, mybir
from concourse.alu_op_type import AluOpType

B, N, D = 32, 2048, 2
NCORES = 8
BL = B // NCORES          # batches per core
NCHUNK = N // 128         # 16 pred-chunks per batch
EPS = 1e-6
LOSS_WEIGHT = 1.0

F16 = mybir.dt.float16
F32 = mybir.dt.float32
INIT_BIG = 1.0e30         # init value for min accumulators (f32)
INF16 = 60000.0           # "+inf" for fp16 tiles

# ---- tunables (engine load balance) ---------------------------------------
# pred-chunk indices whose eviction+rowmin runs fused on DVE (ttr from PSUM).
# NOTE: this walrus build rejects min/max tensor_tensor on GPSIMD (only
# add/mult/subtract lower on Pool), so all min work lives on DVE and the
# eviction stays on ACT: LP optimum is r1=0, GPS used only for prep subs.
R1_CHUNKS = frozenset()
# how many level-0 pair-min ops of the backward tree go to GPSIMD (of 8)
B_GPS_LVL1 = 0


def chamfer_tile_kernel(ctx: ExitStack, tc: tile.TileContext,
                        pred: bass.AP, target: bass.AP, out: bass.AP):
    nc = tc.nc
    MIN = AluOpType.min

    persist = ctx.enter_context(tc.tile_pool(name="persist", bufs=1))

    # [10, 4*2048] fp16 matmul operands (row layout documented below)
    lhs = persist.tile([10, BL * N], F16)   # pred side (stationary)
    rhs = persist.tile([10, BL * N], F16)   # target side (moving)
    # fwd mins cols 0:64 (b*16+c), bwd mins cols 64:128 (64+b*16+q-chunk)
    minall = persist.tile([128, 128], F32)
    eps_ap = persist.tile([128, 1], F32)
    ones128 = persist.tile([128, 1], F32)
    c_one = persist.tile([1, 16], F16)
    c_inv64 = persist.tile([1, 16], F16)
    out_sb = persist.tile([8, 1], F32)

    nc.vector.memset(eps_ap, EPS)
    nc.vector.memset(ones128, 1.0)
    nc.vector.memset(c_one, 1.0)
    nc.vector.memset(c_inv64, 0.015625)   # 2^-6

    # ------------------------------------------------------------------ prep
    with tc.tile_pool(name="prep", bufs=1) as prep:
        # rows 0-3: pred batches, 4-7: target batches; free = [x:2048 | y:2048]
        raw = prep.tile([8, 2 * N], F32)
        for src, prow in ((pred, 0), (target, 4)):
            for cdim in range(2):
                nc.sync.dma_start(
                    out=raw[prow:prow + 4, cdim * N:(cdim + 1) * N],
                    in_=src[:, :, cdim:cdim + 1].rearrange("b n c -> b (n c)"))

        h = prep.tile([8, 2 * N], F16)
        l = prep.tile([8, 2 * N], F32)
        l6 = prep.tile([8, 2 * N], F16)
        h6 = prep.tile([8, 2 * N], F16)
        nc.vector.tensor_copy(out=h, in_=raw)                      # h = fp16(x)
        nc.gpsimd.tensor_tensor(out=l, in0=raw, in1=h, op=AluOpType.subtract)
        nc.vector.tensor_scalar_mul(out=l6, in0=l, scalar1=64.0)   # (x-h)*2^6
        nc.vector.tensor_scalar_mul(out=h6, in0=h, scalar1=0.015625)

        m2h = prep.tile([8, 2 * N], F16)
        m2l6 = prep.tile([8, 2 * N], F16)
        m2h6 = prep.tile([8, 2 * N], F16)
        # compute on all 8 rows (partition starts must be 0/32/64/96);
        # only target rows 4-7 are consumed downstream. On ACT to keep DVE free.
        nc.scalar.mul(out=m2h, in_=h, mul=-2.0)
        nc.scalar.mul(out=m2l6, in_=l6, mul=-2.0)
        nc.scalar.mul(out=m2h6, in_=h6, mul=-2.0)

        sq = prep.tile([8, 2 * N], F32)
        nc.scalar.square(out=sq, in_=raw)
        nrm = prep.tile([8, N], F32)
        nc.vector.tensor_tensor(out=nrm, in0=sq[:, 0:N], in1=sq[:, N:2 * N],
                                op=AluOpType.add)
        nh = prep.tile([8, N], F16)
        nl = prep.tile([8, N], F32)
        nl6 = prep.tile([8, N], F16)
        nc.vector.tensor_copy(out=nh, in_=nrm)
        nc.gpsimd.tensor_tensor(out=nl, in0=nrm, in1=nh, op=AluOpType.subtract)
        nc.vector.tensor_scalar_mul(out=nl6, in0=nl, scalar1=64.0)

        # -------- assemble matmul operands (DMA row copies, cross-partition)
        # K-row pairing (lhs_k * rhs_k summed over k):
        #  0: hp_x      * -2ht_x        3-5: same for y
        #  1: hp_x/64   * -2lt_x*64
        #  2: lp_x*64   * -2ht_x/64
        #  6: nh_p * 1          7: nl6_p * 2^-6
        #  8: 1 * nh_t          9: 2^-6 * nl6_t
        # one DMA per K-row covering all 4 batches (partition -> free flatten);
        # spread issue across HWDGE engine queues to parallelize descriptor gen
        X, Y = slice(0, N), slice(N, 2 * N)
        P, T = slice(0, 4), slice(4, 8)
        row_srcs = [
            (lhs, 0, h[P, X]), (lhs, 1, h6[P, X]), (lhs, 2, l6[P, X]),
            (lhs, 3, h[P, Y]), (lhs, 4, h6[P, Y]), (lhs, 5, l6[P, Y]),
            (lhs, 6, nh[P, :]), (lhs, 7, nl6[P, :]),
            (rhs, 0, m2h[T, X]), (rhs, 1, m2l6[T, X]), (rhs, 2, m2h6[T, X]),
            (rhs, 3, m2h[T, Y]), (rhs, 4, m2l6[T, Y]), (rhs, 5, m2h6[T, Y]),
            (rhs, 8, nh[T, :]), (rhs, 9, nl6[T, :]),
        ]
        dma_engines = [nc.sync, nc.scalar]
        for i, (dst_t, r, src) in enumerate(row_srcs):
            eng = dma_engines[i % len(dma_engines)]
            eng.dma_start(
                out=dst_t[r:r + 1, :].rearrange("p (b n) -> p b n", b=BL),
                in_=src)
        # constant rows (broadcast tiny memset tiles via DMA)
        for dst_t, dst_r, src in ((lhs, 8, c_one), (lhs, 9, c_inv64),
                                  (rhs, 6, c_one), (rhs, 7, c_inv64)):
            bsrc = bass.AP(tensor=src.tensor, offset=src.offset,
                           ap=[[1, 1], [0, (BL * N) // 16], [1, 16]])
            nc.sync.dma_start(
                out=dst_t[dst_r:dst_r + 1, :].rearrange(
                    "p (a c) -> p a c", c=16),
                in_=bsrc)

    # ------------------------------------------------------------- main loop
    psum_pool = ctx.enter_context(tc.tile_pool(name="ps", bufs=2, space="PSUM"))
    ev_pool = ctx.enter_context(tc.tile_pool(name="ev", bufs=5))
    fold_pool = ctx.enter_context(tc.tile_pool(name="fold", bufs=3))
    btree_pool = ctx.enter_context(tc.tile_pool(name="btree", bufs=6))
    trans_pool = ctx.enter_context(tc.tile_pool(name="trans", bufs=2))

    for b in range(BL):
        stack = []          # (level, tile) binary-counter min-tree
        lvl0_gps = 0
        for c in range(NCHUNK):
            ps = psum_pool.tile([128, N], F32, tag="ps")
            wslice = slice(b * N + 128 * c, b * N + 128 * (c + 1))
            for n in range(4):
                nc.tensor.matmul(
                    ps[:, 512 * n:512 * (n + 1)],
                    lhsT=lhs[:, wslice],
                    rhs=rhs[:, b * N + 512 * n: b * N + 512 * (n + 1)],
                    start=True, stop=True)
            ev = ev_pool.tile([128, N], F16, tag="ev")
            col = b * NCHUNK + c
            # ACT evict fp32->fp16, then forward rowmin on DVE as
            # TT-min fold (2x mode) + 1x tensor_reduce.
            # (tensor_tensor_reduce would fuse these but crashes TRN2 here.)
            nc.scalar.copy(out=ev, in_=ps)
            fs = fold_pool.tile([128, N // 2], F16, tag="fold")
            nc.vector.tensor_tensor(out=fs, in0=ev[:, 0:N // 2],
                                    in1=ev[:, N // 2:N], op=MIN)
            nc.vector.tensor_reduce(out=minall[:, col:col + 1], in_=fs,
                                    axis=mybir.AxisListType.X, op=MIN)
            # backward-direction elementwise-min tree over evicted tiles
            t, lvl = ev, 0
            while stack and stack[-1][0] == lvl:
                prev = stack.pop()[1]
                o = btree_pool.tile([128, N], F16, tag="bt")
                if lvl == 0 and lvl0_gps < B_GPS_LVL1:
                    eng = nc.gpsimd
                    lvl0_gps += 1
                else:
                    eng = nc.vector
                eng.tensor_tensor(out=o, in0=prev, in1=t, op=MIN)
                t, lvl = o, lvl + 1
            stack.append((lvl, t))
        root = stack[-1][1]                      # [128, 2048] fp16
        tb = trans_pool.tile([128, NCHUNK, 128], F16, tag="tr")
        nc.sync.dma_start_transpose(out=tb, in_=root)
        tf = trans_pool.tile([128, NCHUNK, 64], F16, tag="trf")
        nc.vector.tensor_tensor(out=tf, in0=tb[:, :, 0:64],
                                in1=tb[:, :, 64:128], op=MIN)
        nc.vector.tensor_reduce(
            out=minall[:, 64 + b * NCHUNK: 64 + (b + 1) * NCHUNK],
            in_=tf, axis=mybir.AxisListType.X, op=MIN)

    # ------------------------------------------------------------- epilogue
    sqv = persist.tile([128, 128], F32)
    nc.scalar.activation(out=sqv, in_=minall,
                         func=mybir.ActivationFunctionType.Sqrt,
                         bias=eps_ap, scale=1.0)
    sums8 = persist.tile([128, 8], F32)
    nc.vector.tensor_reduce(out=sums8,
                            in_=sqv.rearrange("p (g c) -> p g c", g=8),
                            axis=mybir.AxisListType.X, op=AluOpType.add)
    fin = psum_pool.tile([8, 1], F32, tag="ps")
    nc.tensor.matmul(fin, lhsT=sums8, rhs=ones128, start=True, stop=True)
    nc.scalar.copy(out=out_sb, in_=fin)
    nc.sync.dma_start(out=out, in_=out_sb)


def build_nc():
    nc = bacc.Bacc("TRN2", debug=False)
    pred = nc.dram_tensor("pred", [BL, N, D], F32, kind="ExternalInput")
    target = nc.dram_tensor("target", [BL, N, D], F32, kind="ExternalInput")
    out = nc.dram_tensor("out", [8, 1], F32, kind="ExternalOutput")
    with tile.TileContext(nc) as tc:
        with ExitStack() as ctx:
            chamfer_tile_kernel(ctx, tc, pred.ap(), target.ap(), out.ap())
    nc.compile()
    return nc


_NC = None


def _get_nc():
    global _NC
    if _NC is None:
        _NC = build_nc()
    return _NC


def combine_partials(outs):
    """outs: list of 8 arrays [8,1] -> scalar loss (matches reference)."""
    total = 0.0
    for o in outs:
        o = np.asarray(o, dtype=np.float64).reshape(8)
        fwd, bwd = o[0:4], o[4:8]
        total += float(np.sum((fwd + bwd) / N))
    return np.float32(LOSS_WEIGHT * total / B)


def kernel(pred: np.ndarray, target: np.ndarray) -> np.ndarray:
    from concourse.bass_utils import run_bass_kernel_spmd

    pred = np.ascontiguousarray(np.asarray(pred), dtype=np.float32)
    target = np.ascontiguousarray(np.asarray(target), dtype=np.float32)
    assert pred.shape == (B, N, D) and target.shape == (B, N, D)

    nc = _get_nc()
    in_maps = []
    for c in range(NCORES):
        sl = slice(c * BL, (c + 1) * BL)
        in_maps.append({"pred": pred[sl], "target": target[sl]})
    res = run_bass_kernel_spmd(nc, in_maps, core_ids=list(range(NCORES)))
    outs = [r["out"] for r in res.results]
    return combine_partials(outs)


# revision 19
# speedup vs baseline: 5.8529x; 5.8529x over previous
"""Chamfer distance 2D loss — Trainium2 Bass/Tile kernel.

Problem: pred/target [32, 2048, 2] f32. Per batch: 2048x2048 pairwise
squared-distance matrix, bidirectional min + mean, sqrt(sq+eps), mean over
batch -> scalar.

Strategy (per core, 4 of 32 batches, data parallel over 8 cores):
  - sq[i,j] = |p_i|^2 + |t_j|^2 - 2 p.t computed ENTIRELY on the TensorEngine
    as a K=10 fp16 matmul. fp16 hi/lo splitting (with power-of-2 row scaling
    to dodge fp16 denormal flush) keeps ~2^-24 effective input precision, so
    the catastrophic cancellation in the norm expansion is harmless while the
    PE streams at the full 1 col/cycle fp16 rate (fp32 would be 4x slower).
  - sqrt is monotonic => min-reduce the SQUARED distances; sqrt only on the
    reduced [128, 128] tile at the end.
  - Per (batch, pred-chunk) tile [128 preds x 2048 targets] in PSUM:
      * forward rowmin via DVE tensor_tensor_reduce (fold halves + min-accum)
      * PSUM->SBUF fp16 eviction on the Scalar engine (or fused into the
        rowmin via ttr straight from PSUM for a few tiles, to balance load)
      * backward direction: elementwise-min tree over the 16 evicted tiles
        (split between GPSIMD and DVE), then DMA-transpose + free-dim min.
  - Final: sqrt(min+eps) on ACT, row sums on DVE, partition sum via a
    ones-matmul -> [8,1] per-core output (4 fwd sums, 4 bwd sums).
Host sums the 8 cores' partials exactly as the reference mean does.
"""

import os
import sys
from contextlib import ExitStack

import numpy as np

for _p in ("/opt/trn_rl_repo", "/root/.axon_site/_ro/trn_rl_repo"):
    if os.path.isdir(_p) and _p not in sys.path:
        sys.path.insert(0, _p)

import concourse.bass as bass
import concourse.tile as tile
from concourse import bacc, mybir
from concourse.alu_op_type import AluOpType

B, N, D = 32, 2048, 2
NCORES = 8
BL = B // NCORES          # batches per core
NCHUNK = N // 128         # 16 pred-chunks per batch
EPS = 1e-6
LOSS_WEIGHT = 1.0

F16 = mybir.dt.float16
F32 = mybir.dt.float32
INIT_BIG = 1.0e30         # init value for min accumulators (f32)
INF16 = 60000.0           # "+inf" for fp16 tiles

# ---- tunables (engine load balance) ---------------------------------------
# pred-chunk indices whose eviction+rowmin runs fused on DVE (ttr from PSUM).
# NOTE: this walrus build rejects min/max tensor_tensor on GPSIMD (only
# add/mult/subtract lower on Pool), so all min work lives on DVE and the
# eviction stays on ACT: LP optimum is r1=0, GPS used only for prep subs.
R1_CHUNKS = frozenset()
# how many level-0 pair-min ops of the backward tree go to GPSIMD (of 8)
B_GPS_LVL1 = 0


def chamfer_tile_kernel(ctx: ExitStack, tc: tile.TileContext,
                        pred: bass.AP, target: bass.AP, out: bass.AP):
    nc = tc.nc
    MIN = AluOpType.min

    persist = ctx.enter_context(tc.tile_pool(name="persist", bufs=1))

    # [10, 4*2048] fp16 matmul operands (row layout documented below)
    lhs = persist.tile([10, BL * N], F16)   # pred side (stationary)
    rhs = persist.tile([10, BL * N], F16)   # target side (moving)
    # fwd mins cols 0:64 (b*16+c), bwd mins cols 64:128 (64+b*16+q-chunk)
    minall = persist.tile([128, 128], F32)
    eps_ap = persist.tile([128, 1], F32)
    ones128 = persist.tile([128, 1], F32)
    c_one = persist.tile([1, 16], F16)
    c_inv64 = persist.tile([1, 16], F16)
    out_sb = persist.tile([8, 1], F32)

    nc.vector.memset(eps_ap, EPS)
    nc.vector.memset(ones128, 1.0)
    nc.vector.memset(c_one, 1.0)
    nc.vector.memset(c_inv64, 0.015625)   # 2^-6

    # ------------------------------------------------------------------ prep
    with tc.tile_pool(name="prep", bufs=1) as prep:
        # rows 0-3: pred batches, 4-7: target batches; free = [x:2048 | y:2048]
        raw = prep.tile([8, 2 * N], F32)
        for src, prow in ((pred, 0), (target, 4)):
            for cdim in range(2):
                nc.sync.dma_start(
                    out=raw[prow:prow + 4, cdim * N:(cdim + 1) * N],
                    in_=src[:, :, cdim:cdim + 1].rearrange("b n c -> b (n c)"))

        h = prep.tile([8, 2 * N], F16)
        l = prep.tile([8, 2 * N], F32)
        l6 = prep.tile([8, 2 * N], F16)
        h6 = prep.tile([8, 2 * N], F16)
        nc.vector.tensor_copy(out=h, in_=raw)                      # h = fp16(x)
        nc.gpsimd.tensor_tensor(out=l, in0=raw, in1=h, op=AluOpType.subtract)
        nc.vector.tensor_scalar_mul(out=l6, in0=l, scalar1=64.0)   # (x-h)*2^6
        nc.vector.tensor_scalar_mul(out=h6, in0=h, scalar1=0.015625)

        m2h = prep.tile([8, 2 * N], F16)
        m2l6 = prep.tile([8, 2 * N], F16)
        m2h6 = prep.tile([8, 2 * N], F16)
        # compute on all 8 rows (partition starts must be 0/32/64/96);
        # only target rows 4-7 are consumed downstream. On ACT to keep DVE free.
        nc.scalar.mul(out=m2h, in_=h, mul=-2.0)
        nc.scalar.mul(out=m2l6, in_=l6, mul=-2.0)
        nc.scalar.mul(out=m2h6, in_=h6, mul=-2.0)

        sq = prep.tile([8, 2 * N], F32)
        nc.scalar.square(out=sq, in_=raw)
        nrm = prep.tile([8, N], F32)
        nc.vector.tensor_tensor(out=nrm, in0=sq[:, 0:N], in1=sq[:, N:2 * N],
                                op=AluOpType.add)
        nh = prep.tile([8, N], F16)
        nl = prep.tile([8, N], F32)
        nl6 = prep.tile([8, N], F16)
        nc.vector.tensor_copy(out=nh, in_=nrm)
        nc.gpsimd.tensor_tensor(out=nl, in0=nrm, in1=nh, op=AluOpType.subtract)
        nc.vector.tensor_scalar_mul(out=nl6, in0=nl, scalar1=64.0)

        # -------- assemble matmul operands (DMA row copies, cross-partition)
        # K-row pairing (lhs_k * rhs_k summed over k):
        #  0: hp_x      * -2ht_x        3-5: same for y
        #  1: hp_x/64   * -2lt_x*64
        #  2: lp_x*64   * -2ht_x/64
        #  6: nh_p * 1          7: nl6_p * 2^-6
        #  8: 1 * nh_t          9: 2^-6 * nl6_t
        # one DMA per K-row covering all 4 batches (partition -> free flatten);
        # spread issue across HWDGE engine queues to parallelize descriptor gen
        X, Y = slice(0, N), slice(N, 2 * N)
        P, T = slice(0, 4), slice(4, 8)
        row_srcs = [
            (lhs, 0, h[P, X]), (lhs, 1, h6[P, X]), (lhs, 2, l6[P, X]),
            (lhs, 3, h[P, Y]), (lhs, 4, h6[P, Y]), (lhs, 5, l6[P, Y]),
            (lhs, 6, nh[P, :]), (lhs, 7, nl6[P, :]),
            (rhs, 0, m2h[T, X]), (rhs, 1, m2l6[T, X]), (rhs, 2, m2h6[T, X]),
            (rhs, 3, m2h[T, Y]), (rhs, 4, m2l6[T, Y]), (rhs, 5, m2h6[T, Y]),
            (rhs, 8, nh[T, :]), (rhs, 9, nl6[T, :]),
        ]
        dma_engines = [nc.sync, nc.scalar]
        for i, (dst_t, r, src) in enumerate(row_srcs):
            eng = dma_engines[i % len(dma_engines)]
            eng.dma_start(
                out=dst_t[r:r + 1, :].rearrange("p (b n) -> p b n", b=BL),
                in_=src)
        # constant rows (broadcast tiny memset tiles via DMA)
        for dst_t, dst_r, src in ((lhs, 8, c_one), (lhs, 9, c_inv64),
                                  (rhs, 6, c_one), (rhs, 7, c_inv64)):
            bsrc = bass.AP(tensor=src.tensor, offset=src.offset,
                           ap=[[1, 1], [0, (BL * N) // 16], [1, 16]])
            nc.sync.dma_start(
                out=dst_t[dst_r:dst_r + 1, :].rearrange(
                    "p (a c) -> p a c", c=16),
                in_=bsrc)

    # ------------------------------------------------------------- main loop
    psum_pool = ctx.enter_context(tc.tile_pool(name="ps", bufs=2, space="PSUM"))
    ev_pool = ctx.enter_context(tc.tile_pool(name="ev", bufs=5))
    fold_pool = ctx.enter_context(tc.tile_pool(name="fold", bufs=3))
    btree_pool = ctx.enter_context(tc.tile_pool(name="btree", bufs=6))
    trans_pool = ctx.enter_context(tc.tile_pool(name="trans", bufs=2))

    for b in range(BL):
        stack = []          # (level, tile) binary-counter min-tree
        lvl0_gps = 0
        for c in range(NCHUNK):
            ps = psum_pool.tile([128, N], F32, tag="ps")
            wslice = slice(b * N + 128 * c, b * N + 128 * (c + 1))
            for n in range(4):
                nc.tensor.matmul(
                    ps[:, 512 * n:512 * (n + 1)],
                    lhsT=lhs[:, wslice],
                    rhs=rhs[:, b * N + 512 * n: b * N + 512 * (n + 1)],
                    start=True, stop=True)
            ev = ev_pool.tile([128, N], F16, tag="ev")
            col = b * NCHUNK + c
            # ACT evict fp32->fp16, then forward rowmin on DVE as
            # TT-min fold (2x mode) + 1x tensor_reduce.
            # (tensor_tensor_reduce would fuse these but crashes TRN2 here.)
            nc.scalar.copy(out=ev, in_=ps)
            fs = fold_pool.tile([128, N // 2], F16, tag="fold")
            nc.vector.tensor_tensor(out=fs, in0=ev[:, 0:N // 2],
                                    in1=ev[:, N // 2:N], op=MIN)
            nc.vector.tensor_reduce(out=minall[:, col:col + 1], in_=fs,
                                    axis=mybir.AxisListType.X, op=MIN)
            # backward-direction elementwise-min tree over evicted tiles
            t, lvl = ev, 0
            while stack and stack[-1][0] == lvl:
                prev = stack.pop()[1]
                o = btree_pool.tile([128, N], F16, tag="bt")
                if lvl == 0 and lvl0_gps < B_GPS_LVL1:
                    eng = nc.gpsimd
                    lvl0_gps += 1
                else:
                    eng = nc.vector
                eng.tensor_tensor(out=o, in0=prev, in1=t, op=MIN)
                t, lvl = o, lvl + 1
            stack.append((lvl, t))
        root = stack[-1][1]                      # [128, 2048] fp16
        tb = trans_pool.tile([128, NCHUNK, 128], F16, tag="tr")
        nc.sync.dma_start_transpose(out=tb, in_=root)
        tf = trans_pool.tile([128, NCHUNK, 64], F16, tag="trf")
        nc.vector.tensor_tensor(out=tf, in0=tb[:, :, 0:64],
                                in1=tb[:, :, 64:128], op=MIN)
        nc.vector.tensor_reduce(
            out=minall[:, 64 + b * NCHUNK: 64 + (b + 1) * NCHUNK],
            in_=tf, axis=mybir.AxisListType.X, op=MIN)

    # ------------------------------------------------------------- epilogue
    sqv = persist.tile([128, 128], F32)
    nc.scalar.activation(out=sqv, in_=minall,
                         func=mybir.ActivationFunctionType.Sqrt,
                         bias=eps_ap, scale=1.0)
    sums8 = persist.tile([128, 8], F32)
    nc.vector.tensor_reduce(out=sums8,
                            in_=sqv.rearrange("p (g c) -> p g c", g=8),
                            axis=mybir.AxisListType.X, op=AluOpType.add)
    fin = psum_pool.tile([8, 1], F32, tag="ps")
    nc.tensor.matmul(fin, lhsT=sums8, rhs=ones128, start=True, stop=True)
    nc.scalar.copy(out=out_sb, in_=fin)
    nc.sync.dma_start(out=out, in_=out_sb)


def build_nc():
    nc = bacc.Bacc("TRN2", debug=False)
    pred = nc.dram_tensor("pred", [BL, N, D], F32, kind="ExternalInput")
    target = nc.dram_tensor("target", [BL, N, D], F32, kind="ExternalInput")
    out = nc.dram_tensor("out", [8, 1], F32, kind="ExternalOutput")
    with tile.TileContext(nc) as tc:
        with ExitStack() as ctx:
            chamfer_tile_kernel(ctx, tc, pred.ap(), target.ap(), out.ap())
    nc.compile()
    return nc


_NC = None


def _get_nc():
    global _NC
    if _NC is None:
        _NC = build_nc()
    return _NC


def combine_partials(outs):
    """outs: list of 8 arrays [8,1] -> scalar loss (matches reference)."""
    total = 0.0
    for o in outs:
        o = np.asarray(o, dtype=np.float64).reshape(8)
        fwd, bwd = o[0:4], o[4:8]
        total += float(np.sum((fwd + bwd) / N))
    return np.float32(LOSS_WEIGHT * total / B)


_RUNNER = None


def _get_runner():
    """Cached jitted 8-core executor (run_bass_via_pjrt re-traces per call;
    this builds the shard_map once and reuses it)."""
    global _RUNNER
    if _RUNNER is not None:
        return _RUNNER
    import jax
    from jax.sharding import Mesh, PartitionSpec
    try:
        from jax.experimental.shard_map import shard_map
    except Exception:
        from jax.shard_map import shard_map  # newer jax
    from concourse import bass2jax
    from concourse.bass2jax import _bass_exec_p, install_neuronx_cc_hook

    install_neuronx_cc_hook()
    nc = _get_nc()

    in_names, out_names, out_avals = [], [], []
    for alloc in nc.m.functions[0].allocations:
        if not isinstance(alloc, mybir.MemoryLocationSet):
            continue
        name = alloc.memorylocations[0].name
        if alloc.kind == "ExternalInput":
            if nc.partition_id_tensor is None or \
                    name != nc.partition_id_tensor.name:
                in_names.append(name)
        elif alloc.kind == "ExternalOutput":
            out_names.append(name)
            out_avals.append(jax.core.ShapedArray(
                tuple(alloc.tensor_shape), mybir.dt.np(alloc.dtype)))
    n_params = len(in_names)
    all_in_names = list(in_names) + list(out_names)
    if nc.partition_id_tensor is not None:
        all_in_names.append(nc.partition_id_tensor.name)

    def _body(*args):
        operands = list(args)
        if nc.partition_id_tensor is not None:
            operands.append(bass2jax.partition_id_tensor())
        return tuple(_bass_exec_p.bind(
            *operands,
            out_avals=tuple(out_avals),
            in_names=tuple(all_in_names),
            out_names=tuple(out_names),
            lowering_input_output_aliases=(),
            sim_require_finite=True,
            sim_require_nnan=True,
            nc=nc,
        ))

    devices = jax.devices()[:NCORES]
    mesh = Mesh(np.asarray(devices), ("core",))
    n_outs = len(out_names)
    sharded = jax.jit(
        shard_map(_body, mesh=mesh,
                  in_specs=(PartitionSpec("core"),) * (n_params + n_outs),
                  out_specs=(PartitionSpec("core"),) * n_outs,
                  check_rep=False),
        keep_unused=True,
    )
    zero_outs = [np.zeros((NCORES * a.shape[0], *a.shape[1:]), a.dtype)
                 for a in out_avals]

    def run(pred, target):
        ins = {"pred": pred, "target": target}
        concat_in = [ins[nm] for nm in in_names]
        out_arrs = sharded(*concat_in, *zero_outs)
        o = np.asarray(out_arrs[out_names.index("out")])
        return o.reshape(NCORES, 8, 1)

    _RUNNER = run
    return _RUNNER


def kernel(pred: np.ndarray, target: np.ndarray) -> np.ndarray:
    pred = np.ascontiguousarray(np.asarray(pred), dtype=np.float32)
    target = np.ascontiguousarray(np.asarray(target), dtype=np.float32)
    assert pred.shape == (B, N, D) and target.shape == (B, N, D)
    run = _get_runner()
    outs = run(pred, target)
    return combine_partials(list(outs))
